# revision 3
# baseline (speedup 1.0000x reference)
"""TRN2 Bass kernel for nn_KVGather: out[b,i,t] = kv[b, r_idx[b,i,t]] * r_weight[b,i,t].

Full shapes: r_idx/r_weight (32,49,4), kv (32,49,64,256) f32 -> out (32,49,4,64,256) f32.

Sharding: batch dim n=32 across 8 cores (4 batches/core), pure data parallel.

Per-core device kernel (memory-bound, bf16 I/O):
  - Gather+scale is a one-hot matmul: out_tile_chunk[98, 512] =
    W_b[49, 98].T @ kv_b[49, 512] where W_b[r, j] = r_weight[j] iff
    r == r_idx[j] (host-built, bf16). Indices are batch-local (p2=49 regions)
    so contraction is only 49 rows. No dynamic APs, no register loads.
  - kv resident in SBUF as [49 partitions, 4 batches, 16384] bf16 (128 KB/part).
  - PE: 8 tile-chunks x 32 psum-bank matmuls = 256 matmuls of [49,98]x[49,512].
  - PSUM f32 -> SBUF bf16 evictions in [98, 1024] (2-bank) chunks, split
    across Activation/DVE/Pool (7/5/4 per 16) to keep pace with DMA.
  - Output written as natural row-major (784, 16384) bf16, 32 KB descriptors
    (full DMA bus bandwidth); host upcasts to f32.
"""

import os
import sys

sys.path.insert(0, "/opt/trn_rl_repo")

import numpy as np

N, P2, TOPK, HW_KV, C_KV = 32, 49, 4, 64, 256
NCORES = 8
NB = N // NCORES  # 4 batches per core
ROW = HW_KV * C_KV  # 16384 elems per kv row / output tile
TPB = P2 * TOPK  # 196 output tiles per batch
TILES = NB * TPB  # 784 output tiles per core
TCH = 98  # tiles per psum chunk (2 chunks per batch)
ECH = 512  # elems per matmul (= one psum bank of f32)
EV = 1024  # elems per eviction op (2 psum banks)
NEV = ROW // EV  # 16 evictions per (batch, chunk)

# eviction engine pattern per 16 evictions: ACT x9, DVE x7
# (GPSIMD cannot access PSUM on TRN2; proportional to modeled op rates)
_EV_PAT = "ADADADADADADADAA"

_compiled = None


def _build():
    import concourse.bass as bass
    import concourse.tile as tile
    from concourse import bacc, mybir

    nc = bacc.Bacc("TRN2", target_bir_lowering=False, debug=False)

    f32 = mybir.dt.float32
    bf16 = mybir.dt.bfloat16
    COPY = mybir.ActivationFunctionType.Copy

    kv_d = nc.dram_tensor("kv", [P2, NB, ROW], bf16, kind="ExternalInput").ap()
    w_d = nc.dram_tensor("w", [P2, TILES], bf16, kind="ExternalInput").ap()
    out_d = nc.dram_tensor("out", [TILES, ROW], bf16, kind="ExternalOutput").ap()

    with tile.TileContext(nc) as tc:
        with (
            tc.tile_pool(name="res", bufs=1) as res_pool,
            tc.tile_pool(name="stage", bufs=2) as stage_pool,
            tc.tile_pool(name="psum", bufs=4, space=bass.MemorySpace.PSUM) as psum_pool,
        ):
            kv_sb = res_pool.tile([P2, NB, ROW], bf16, tag="kv")
            w_sb = res_pool.tile([P2, TILES], bf16, tag="w")

            nc.sync.dma_start(w_sb[:], w_d[:])
            for b in range(NB):
                nc.sync.dma_start(kv_sb[:, b, :], kv_d[:, b, :])

            for b in range(NB):
                for tcx in range(2):
                    j0 = b * TPB + tcx * TCH
                    stage = stage_pool.tile([TCH, ROW], bf16, tag="st")
                    for ev in range(NEV):
                        ps = psum_pool.tile([TCH, EV], f32, tag="ps")
                        for h in range(2):
                            e0 = ev * EV + h * ECH
                            nc.tensor.matmul(
                                ps[:, h * ECH : (h + 1) * ECH],
                                w_sb[:, j0 : j0 + TCH],
                                kv_sb[:, b, e0 : e0 + ECH],
                                start=True,
                                stop=True,
                            )
                        dst = stage[:, ev * EV : (ev + 1) * EV]
                        eng = _EV_PAT[ev]
                        if eng == "A":
                            nc.scalar.activation(dst, ps[:], COPY)
                        elif eng == "D":
                            nc.vector.tensor_copy(dst, ps[:])
                        else:
                            nc.gpsimd.tensor_copy(dst, ps[:])

                    nc.sync.dma_start(out_d[j0 : j0 + TCH, :], stage[:])

    nc.compile()
    return nc


def _get_compiled():
    global _compiled
    if _compiled is None:
        _compiled = _build()
    return _compiled


def _enable_trace_hook():
    """Register the axon NTFF profile hook (missing antenv.axon_hooks shim)."""
    import types

    try:
        import antenv.axon_hooks  # noqa: F401

        return
    except ImportError:
        pass
    try:
        import antenv

        mod = types.ModuleType("antenv.axon_hooks")
        holder = {}
        mod.set_axon_ntff_profile_hook = lambda h: holder.__setitem__("h", h)
        mod.get_axon_ntff_profile_hook = lambda: holder.get("h")
        antenv.axon_hooks = mod
        sys.modules["antenv.axon_hooks"] = mod
        if "/root/.axon_site" not in sys.path:
            sys.path.insert(0, "/root/.axon_site")
        from trn_agent_boot.trn_boot import _ntff_profile_via_ctypes

        mod.set_axon_ntff_profile_hook(
            _ntff_profile_via_ctypes("/opt/axon/libaxon_pjrt.so")
        )

        import concourse.bass_utils as bu

        orig = bu.upload_artifacts

        def _safe_upload(tmpdir):
            try:
                return orig(tmpdir)
            except Exception:
                return tmpdir
    except Exception as e:  # tracing is best-effort
        print(f"trace hook setup failed: {e}")


def kernel(r_idx, r_weight, kv):
    import ml_dtypes

    from concourse.bass_utils import run_bass_kernel_spmd

    bf16 = ml_dtypes.bfloat16

    r_idx = np.asarray(r_idx)
    r_weight = np.asarray(r_weight, dtype=np.float32)
    kv = np.asarray(kv, dtype=np.float32)
    assert r_idx.shape == (N, P2, TOPK) and kv.shape == (N, P2, HW_KV, C_KV)

    nc = _get_compiled()

    in_maps = []
    for c in range(NCORES):
        b0 = c * NB
        kv_c = kv[b0 : b0 + NB].reshape(NB, P2, ROW)
        kvT = np.ascontiguousarray(kv_c.transpose(1, 0, 2)).astype(bf16)
        idx = np.asarray(r_idx[b0 : b0 + NB], dtype=np.int64).reshape(NB * TPB)
        wv = r_weight[b0 : b0 + NB].reshape(NB * TPB)
        W = np.zeros((P2, TILES), dtype=np.float32)
        W[idx, np.arange(TILES)] = wv
        in_maps.append({"kv": kvT, "w": W.astype(bf16)})

    trace = bool(int(os.environ.get("KV_TRACE", "0")))
    if trace:
        _enable_trace_hook()
    res = run_bass_kernel_spmd(nc, in_maps, list(range(NCORES)), trace=trace)

    if trace:
        kernel.last_exec_time_ns = res.exec_time_ns
        kernel.last_trace = (
            res.instructions_and_trace[1] if res.instructions_and_trace else None
        )

    out = np.empty((N, P2, TOPK, HW_KV, C_KV), dtype=np.float32)
    for c in range(NCORES):
        b0 = c * NB
        out[b0 : b0 + NB] = (
            np.asarray(res.results[c]["out"])
            .astype(np.float32)
            .reshape(NB, P2, TOPK, HW_KV, C_KV)
        )
    return out


# revision 6
# speedup vs baseline: 1.0101x; 1.0101x over previous
"""TRN2 Bass kernel for nn_KVGather: out[b,i,t] = kv[b, r_idx[b,i,t]] * r_weight[b,i,t].

Full shapes: r_idx/r_weight (32,49,4), kv (32,49,64,256) f32 -> out (32,49,4,64,256) f32.

Sharding: batch dim n=32 across 8 cores (4 batches/core), pure data parallel.

Per-core device kernel (memory-bound, bf16 I/O):
  - Gather+scale is a one-hot matmul: out_tile_chunk[98, 512] =
    W_b[49, 98].T @ kv_b[49, 512] where W_b[r, j] = r_weight[j] iff
    r == r_idx[j] (host-built, bf16). Indices are batch-local (p2=49 regions)
    so contraction is only 49 rows. No dynamic APs, no register loads.
  - kv resident in SBUF as 8 half-row tiles [49, 8192] bf16 so the first
    matmuls only wait on the first 1.6 MB of DMA; loaded on the gpsimd
    (SWDGE) queue so output DMAs on the sync (HWDGE) queue are not
    head-blocked behind them.
  - PE: 8 tile-chunks x 32 psum-bank matmuls = 256 matmuls of [49,98]x[49,512].
  - PSUM f32 -> SBUF bf16 evictions in [98, 1024] (2-bank) chunks, split
    ACT 9 / DVE 7 per 16 (GPSIMD cannot access PSUM on TRN2).
  - Output written as natural row-major (784, 16384) bf16 in half-row DMAs
    (16 KB descriptors, full DMA bus bandwidth); host upcasts to f32.
"""

import os
import sys

sys.path.insert(0, "/opt/trn_rl_repo")

import numpy as np

N, P2, TOPK, HW_KV, C_KV = 32, 49, 4, 64, 256
NCORES = 8
NB = N // NCORES  # 4 batches per core
ROW = HW_KV * C_KV  # 16384 elems per kv row / output tile
HROW = ROW // 2  # half row
TPB = P2 * TOPK  # 196 output tiles per batch
TILES = NB * TPB  # 784 output tiles per core
TCH = 98  # tiles per psum chunk (2 chunks per batch)
ECH = 512  # elems per matmul (= one psum bank of f32)
EV = 1024  # elems per eviction op (2 psum banks)
NEV = ROW // EV  # 16 evictions per (batch, chunk)

# eviction engine pattern per 16 evictions: ACT x9, DVE x7
# (GPSIMD cannot access PSUM on TRN2; proportional to modeled op rates)
_EV_PAT = "ADADADADADADADAA"

_compiled = None


def _build():
    import concourse.bass as bass
    import concourse.tile as tile
    from concourse import bacc, mybir

    nc = bacc.Bacc("TRN2", target_bir_lowering=False, debug=False)

    f32 = mybir.dt.float32
    bf16 = mybir.dt.bfloat16
    COPY = mybir.ActivationFunctionType.Copy

    # kv halves: kvh_d[b, h] = rows [49, 8192] of batch b, row half h
    kv_d = nc.dram_tensor("kv", [P2, NB, 2, HROW], bf16, kind="ExternalInput").ap()
    w_d = nc.dram_tensor("w", [P2, TILES], bf16, kind="ExternalInput").ap()
    out_d = nc.dram_tensor("out", [TILES, ROW], bf16, kind="ExternalOutput").ap()

    with tile.TileContext(nc) as tc:
        with (
            tc.tile_pool(name="res", bufs=1) as res_pool,
            tc.tile_pool(name="kvp", bufs=6) as kv_pool,
            tc.tile_pool(name="stage", bufs=6) as stage_pool,
            tc.tile_pool(name="psum", bufs=4, space=bass.MemorySpace.PSUM) as psum_pool,
        ):
            w_sb = res_pool.tile([P2, TILES], bf16, tag="w")
            nc.gpsimd.dma_start(w_sb[:], w_d[:])

            kvh = {}

            def load_kv(b):
                for h in range(2):
                    t = kv_pool.tile([P2, HROW], bf16, tag="kv")
                    nc.gpsimd.dma_start(t[:], kv_d[:, b, h, :])
                    kvh[b, h] = t

            for b in range(3):
                load_kv(b)

            for b in range(NB):
                if b == 2:
                    load_kv(3)  # reuses batch-0's buffers (pool bufs=6)
                for tcx in range(2):
                    j0 = b * TPB + tcx * TCH
                    for sh in range(2):  # output half-rows, each its own tile
                        stage = stage_pool.tile([TCH, HROW], bf16, tag="st")
                        for evh in range(NEV // 2):
                            ev = sh * (NEV // 2) + evh
                            ps = psum_pool.tile([TCH, EV], f32, tag="ps")
                            for hh in range(2):
                                ec = ev * 2 + hh
                                src = kvh[b, ec // 16]
                                e0 = (ec % 16) * ECH
                                nc.tensor.matmul(
                                    ps[:, hh * ECH : (hh + 1) * ECH],
                                    w_sb[:, j0 : j0 + TCH],
                                    src[:, e0 : e0 + ECH],
                                    start=True,
                                    stop=True,
                                )
                            dst = stage[:, evh * EV : (evh + 1) * EV]
                            if _EV_PAT[ev] == "A":
                                nc.scalar.activation(dst, ps[:], COPY)
                            else:
                                nc.vector.tensor_copy(dst, ps[:])
                        nc.sync.dma_start(
                            out_d[j0 : j0 + TCH, sh * HROW : (sh + 1) * HROW],
                            stage[:],
                        )

    nc.compile()
    return nc


def _get_compiled():
    global _compiled
    if _compiled is None:
        _compiled = _build()
    return _compiled


def _enable_trace_hook():
    """Register the axon NTFF profile hook (missing antenv.axon_hooks shim)."""
    import types

    try:
        import antenv.axon_hooks  # noqa: F401

        return
    except ImportError:
        pass
    try:
        import antenv

        mod = types.ModuleType("antenv.axon_hooks")
        holder = {}
        mod.set_axon_ntff_profile_hook = lambda h: holder.__setitem__("h", h)
        mod.get_axon_ntff_profile_hook = lambda: holder.get("h")
        antenv.axon_hooks = mod
        sys.modules["antenv.axon_hooks"] = mod
        if "/root/.axon_site" not in sys.path:
            sys.path.insert(0, "/root/.axon_site")
        from trn_agent_boot.trn_boot import _ntff_profile_via_ctypes

        mod.set_axon_ntff_profile_hook(
            _ntff_profile_via_ctypes("/opt/axon/libaxon_pjrt.so")
        )

        import concourse.bass_utils as bu

        orig = bu.upload_artifacts

        def _safe_upload(tmpdir):
            try:
                return orig(tmpdir)
            except Exception:
                return tmpdir
    except Exception as e:  # tracing is best-effort
        print(f"trace hook setup failed: {e}")


def kernel(r_idx, r_weight, kv):
    import ml_dtypes

    from concourse.bass_utils import run_bass_kernel_spmd

    bf16 = ml_dtypes.bfloat16

    r_idx = np.asarray(r_idx)
    r_weight = np.asarray(r_weight, dtype=np.float32)
    kv = np.asarray(kv, dtype=np.float32)
    assert r_idx.shape == (N, P2, TOPK) and kv.shape == (N, P2, HW_KV, C_KV)

    nc = _get_compiled()

    in_maps = []
    for c in range(NCORES):
        b0 = c * NB
        kv_c = kv[b0 : b0 + NB].reshape(NB, P2, 2, HROW)
        kvT = np.ascontiguousarray(kv_c.transpose(1, 0, 2, 3)).astype(bf16)
        idx = np.asarray(r_idx[b0 : b0 + NB], dtype=np.int64).reshape(NB * TPB)
        wv = r_weight[b0 : b0 + NB].reshape(NB * TPB)
        W = np.zeros((P2, TILES), dtype=np.float32)
        W[idx, np.arange(TILES)] = wv
        in_maps.append({"kv": kvT, "w": W.astype(bf16)})

    trace = bool(int(os.environ.get("KV_TRACE", "0")))
    if trace:
        _enable_trace_hook()
    res = run_bass_kernel_spmd(nc, in_maps, list(range(NCORES)), trace=trace)

    if trace:
        kernel.last_exec_time_ns = res.exec_time_ns
        kernel.last_trace = (
            res.instructions_and_trace[1] if res.instructions_and_trace else None
        )

    out = np.empty((N, P2, TOPK, HW_KV, C_KV), dtype=np.float32)
    for c in range(NCORES):
        b0 = c * NB
        out[b0 : b0 + NB] = (
            np.asarray(res.results[c]["out"])
            .astype(np.float32)
            .reshape(NB, P2, TOPK, HW_KV, C_KV)
        )
    return out


# revision 7
# speedup vs baseline: 1.2778x; 1.2650x over previous
"""TRN2 Bass kernel for nn_KVGather: out[b,i,t] = kv[b, r_idx[b,i,t]] * r_weight[b,i,t].

Full shapes: r_idx/r_weight (32,49,4), kv (32,49,64,256) f32 -> out (32,49,4,64,256) f32.

Sharding: batch dim n=32 across 8 cores (4 batches/core), pure data parallel.

Per-core device kernel (memory-bound, bf16 I/O):
  - Gather+scale as a one-hot matmul with the *kv element-slice* stationary:
      psum[128 elems, 392 tiles] = kv2[98, 128].T @ W2[98, 392]
    where kv2 stacks the rows of a batch PAIR on 98 partitions (indices are
    batch-local) and W2[r, j] = r_weight[j] one-hot in r. 256 matmuls of
    392 moving columns — ~40% less PE time than tile-stationary chunks, and
    no dynamic APs or register loads.
  - PSUM f32 -> SBUF bf16 evictions [128, 392], alternating ACT/DVE
    (GPSIMD cannot access PSUM on TRN2).
  - Output DRAM layout is the blocked [pair, ec, e, j] transpose (784 B
    descriptor lines); the host unpermutes with numpy for free.
  - All DMAs on the sync/HWDGE queue (SWDGE runs at half per-engine rate);
    kv pair loads are interleaved between output DMAs to avoid head-blocking.
"""

import os
import sys

sys.path.insert(0, "/opt/trn_rl_repo")

import numpy as np

N, P2, TOPK, HW_KV, C_KV = 32, 49, 4, 64, 256
NCORES = 8
NB = N // NCORES  # 4 batches per core
ROW = HW_KV * C_KV  # 16384 elems per kv row / output tile
HROW = ROW // 2  # 8192, kv row half held per (pair, half) SBUF tile
TPB = P2 * TOPK  # 196 output tiles per batch
TILES = NB * TPB  # 784 output tiles per core
NPAIR = 2  # batch pairs (0,1) and (2,3)
CP = 2 * P2  # 98 contraction rows per pair
MT = 2 * TPB  # 392 moving columns (= tiles of one pair)
EC = 128  # elems per matmul (stationary free dim)
NECH = HROW // EC  # 64 e-chunks per (pair, half)
KG = 8  # e-chunks per stage buffer / output DMA

# eviction engine split ACT:DVE proportional to modeled op rates
_N_ACT = 137  # of 256 total evictions


def _ev_engine(i):
    return "A" if (i + 1) * _N_ACT // 256 - i * _N_ACT // 256 else "D"


_compiled = None


def _build():
    import concourse.bass as bass
    import concourse.tile as tile
    from concourse import bacc, mybir

    nc = bacc.Bacc("TRN2", target_bir_lowering=False, debug=False)

    f32 = mybir.dt.float32
    bf16 = mybir.dt.bfloat16
    COPY = mybir.ActivationFunctionType.Copy

    kv_d = nc.dram_tensor("kv", [CP, NPAIR, 2, HROW], bf16, kind="ExternalInput").ap()
    w_d = nc.dram_tensor("w", [CP, NPAIR, MT], bf16, kind="ExternalInput").ap()
    out_d = nc.dram_tensor("out", [NPAIR, ROW // EC, EC, MT], bf16, kind="ExternalOutput").ap()
    # out viewed per DMA group: [pair, group, e-part, k-in-group, j]
    out_v = out_d.rearrange("g (kb k) e j -> g kb e k j", k=KG)

    with tile.TileContext(nc) as tc:
        with (
            tc.tile_pool(name="res", bufs=1) as res_pool,
            tc.tile_pool(name="kvp", bufs=4) as kv_pool,
            tc.tile_pool(name="stage", bufs=4) as stage_pool,
            tc.tile_pool(name="psum", bufs=8, space=bass.MemorySpace.PSUM) as psum_pool,
        ):
            w_sb = res_pool.tile([CP, NPAIR, MT], bf16, tag="w")
            nc.sync.dma_start(w_sb[:], w_d[:])

            kvh = {}

            def load_kv(g, h):
                t = kv_pool.tile([CP, HROW], bf16, tag="kv")
                nc.sync.dma_start(t[:], kv_d[:, g, h, :])
                kvh[g, h] = t

            load_kv(0, 0)
            load_kv(0, 1)

            ev_i = 0
            unit = 0
            for g in range(NPAIR):
                for h in range(2):
                    for kb in range(NECH // KG):
                        stage = stage_pool.tile([EC, KG * MT], bf16, tag="st")
                        for kk in range(KG):
                            ecl = kb * KG + kk
                            ps = psum_pool.tile([EC, MT], f32, tag="ps")
                            nc.tensor.matmul(
                                ps[:],
                                kvh[g, h][:, ecl * EC : (ecl + 1) * EC],
                                w_sb[:, g, :],
                                start=True,
                                stop=True,
                            )
                            dst = stage[:, kk * MT : (kk + 1) * MT]
                            if _ev_engine(ev_i) == "A":
                                nc.scalar.activation(dst, ps[:], COPY)
                            else:
                                nc.vector.tensor_copy(dst, ps[:])
                            ev_i += 1
                        nc.sync.dma_start(
                            out_v[g, h * (NECH // KG) + kb],
                            stage[:].rearrange("e (k j) -> e k j", j=MT),
                        )
                        unit += 1
                        if unit == 4:
                            load_kv(1, 0)
                        elif unit == 8:
                            load_kv(1, 1)

    nc.compile()
    return nc


def _get_compiled():
    global _compiled
    if _compiled is None:
        _compiled = _build()
    return _compiled


def _enable_trace_hook():
    """Register the axon NTFF profile hook (missing antenv.axon_hooks shim)."""
    import types

    try:
        import antenv.axon_hooks  # noqa: F401

        return
    except ImportError:
        pass
    try:
        import antenv

        mod = types.ModuleType("antenv.axon_hooks")
        holder = {}
        mod.set_axon_ntff_profile_hook = lambda h: holder.__setitem__("h", h)
        mod.get_axon_ntff_profile_hook = lambda: holder.get("h")
        antenv.axon_hooks = mod
        sys.modules["antenv.axon_hooks"] = mod
        if "/root/.axon_site" not in sys.path:
            sys.path.insert(0, "/root/.axon_site")
        from trn_agent_boot.trn_boot import _ntff_profile_via_ctypes

        mod.set_axon_ntff_profile_hook(
            _ntff_profile_via_ctypes("/opt/axon/libaxon_pjrt.so")
        )

        import concourse.bass_utils as bu

        orig = bu.upload_artifacts

        def _safe_upload(tmpdir):
            try:
                return orig(tmpdir)
            except Exception:
                return tmpdir
    except Exception as e:  # tracing is best-effort
        print(f"trace hook setup failed: {e}")


def kernel(r_idx, r_weight, kv):
    import ml_dtypes

    from concourse.bass_utils import run_bass_kernel_spmd

    bf16 = ml_dtypes.bfloat16

    r_idx = np.asarray(r_idx)
    r_weight = np.asarray(r_weight, dtype=np.float32)
    kv = np.asarray(kv, dtype=np.float32)
    assert r_idx.shape == (N, P2, TOPK) and kv.shape == (N, P2, HW_KV, C_KV)

    nc = _get_compiled()

    cols = np.arange(TPB)
    in_maps = []
    for c in range(NCORES):
        b0 = c * NB
        # kv2[bip*49 + row, g, h, e] = kv[b0 + 2g + bip, row, h*8192 + e]
        kv_c = kv[b0 : b0 + NB].reshape(NPAIR, 2, P2, 2, HROW)
        kvT2 = np.ascontiguousarray(kv_c.transpose(1, 2, 0, 3, 4)).reshape(
            CP, NPAIR, 2, HROW
        )
        idx4 = np.asarray(r_idx[b0 : b0 + NB], dtype=np.int64).reshape(NB, TPB)
        w4 = r_weight[b0 : b0 + NB].reshape(NB, TPB)
        W2 = np.zeros((CP, NPAIR, MT), dtype=np.float32)
        for g in range(NPAIR):
            for bip in range(2):
                b = 2 * g + bip
                W2[bip * P2 + idx4[b], g, bip * TPB + cols] = w4[b]
        in_maps.append({"kv": kvT2.astype(bf16), "w": W2.astype(bf16)})

    trace = bool(int(os.environ.get("KV_TRACE", "0")))
    if trace:
        _enable_trace_hook()
    res = run_bass_kernel_spmd(nc, in_maps, list(range(NCORES)), trace=trace)

    if trace:
        kernel.last_exec_time_ns = res.exec_time_ns
        kernel.last_trace = (
            res.instructions_and_trace[1] if res.instructions_and_trace else None
        )

    out = np.empty((N, P2, TOPK, HW_KV, C_KV), dtype=np.float32)
    for c in range(NCORES):
        b0 = c * NB
        a = np.asarray(res.results[c]["out"]).reshape(NPAIR, ROW // EC, EC, 2, TPB)
        a = a.transpose(0, 3, 4, 1, 2).reshape(NB, TPB, ROW)
        out[b0 : b0 + NB] = a.astype(np.float32).reshape(NB, P2, TOPK, HW_KV, C_KV)
    return out


# revision 10
# speedup vs baseline: 1.3564x; 1.0615x over previous
"""TRN2 Bass kernel for nn_KVGather: out[b,i,t] = kv[b, r_idx[b,i,t]] * r_weight[b,i,t].

Full shapes: r_idx/r_weight (32,49,4), kv (32,49,64,256) f32 -> out (32,49,4,64,256) f32.

Sharding: batch dim n=32 across 8 cores (4 batches/core), pure data parallel.

Per-core device kernel (memory-bound, bf16 I/O):
  - Gather+scale as a one-hot matmul with the *kv element-slice* stationary:
      psum[128 elems, 392 tiles] = kv2[98, 128].T @ W2[98, 392]
    where kv2 stacks the rows of a batch PAIR on 98 partitions (indices are
    batch-local) and W2[r, j] = r_weight[j] one-hot in r. 256 matmuls of
    392 moving columns — ~40% less PE time than tile-stationary chunks, and
    no dynamic APs or register loads.
  - PSUM f32 -> SBUF bf16 evictions [128, 392], alternating ACT/DVE
    (GPSIMD cannot access PSUM on TRN2).
  - Output DRAM layout is the blocked [pair, ec, e, j] transpose (784 B
    descriptor lines); the host unpermutes with numpy for free.
  - All DMAs on the sync/HWDGE queue (SWDGE runs at half per-engine rate);
    kv pair loads are interleaved between output DMAs to avoid head-blocking.
"""

import os
import sys

sys.path.insert(0, "/opt/trn_rl_repo")

import numpy as np

N, P2, TOPK, HW_KV, C_KV = 32, 49, 4, 64, 256
NCORES = 8
NB = N // NCORES  # 4 batches per core
ROW = HW_KV * C_KV  # 16384 elems per kv row / output tile
HROW = ROW // 2  # 8192, kv row half held per (pair, half) SBUF tile
TPB = P2 * TOPK  # 196 output tiles per batch
TILES = NB * TPB  # 784 output tiles per core
NPAIR = 2  # batch pairs (0,1) and (2,3)
CP = 2 * P2  # 98 contraction rows per pair
MT = 2 * TPB  # 392 moving columns (= tiles of one pair)
EC = 128  # elems per matmul (stationary free dim)
NECH = HROW // EC  # 64 e-chunks per (pair, half)
KG = 8  # e-chunks per stage buffer / output DMA

# eviction engine split ACT:DVE proportional to modeled op rates
_N_ACT = 137  # of 256 total evictions


def _ev_engine(i):
    return "A" if (i + 1) * _N_ACT // 256 - i * _N_ACT // 256 else "D"


_compiled = None


def _build():
    import concourse.bass as bass
    import concourse.tile as tile
    from concourse import bacc, mybir

    nc = bacc.Bacc("TRN2", target_bir_lowering=False, debug=False)

    f32 = mybir.dt.float32
    bf16 = mybir.dt.bfloat16
    COPY = mybir.ActivationFunctionType.Copy

    kv_d = nc.dram_tensor("kv", [CP, NPAIR, 2, HROW], bf16, kind="ExternalInput").ap()
    w_d = nc.dram_tensor("w", [CP, NPAIR, MT], bf16, kind="ExternalInput").ap()
    out_d = nc.dram_tensor("out", [NPAIR, ROW // EC, EC, MT], bf16, kind="ExternalOutput").ap()
    # out viewed per DMA group: [pair, group, e-part, k-in-group, j]
    out_v = out_d.rearrange("g (kb k) e j -> g kb e k j", k=KG)

    with tile.TileContext(nc) as tc:
        with (
            tc.tile_pool(name="res", bufs=1) as res_pool,
            tc.tile_pool(name="kvp", bufs=4) as kv_pool,
            tc.tile_pool(name="stage", bufs=6) as stage_pool,
            tc.tile_pool(name="psum", bufs=8, space=bass.MemorySpace.PSUM) as psum_pool,
        ):
            w_sb = res_pool.tile([CP, NPAIR, MT], bf16, tag="w")
            nc.sync.dma_start(w_sb[:], w_d[:])

            kvh = {}

            def load_kv(g, h):
                t = kv_pool.tile([CP, HROW], bf16, tag="kv")
                nc.sync.dma_start(t[:], kv_d[:, g, h, :])
                kvh[g, h] = t

            load_kv(0, 0)

            ev_i = 0
            unit = 0
            for g in range(NPAIR):
                for h in range(2):
                    for kb in range(NECH // KG):
                        stage = stage_pool.tile([EC, KG * MT], bf16, tag="st")
                        for kk in range(KG):
                            ecl = kb * KG + kk
                            ps = psum_pool.tile([EC, MT], f32, tag="ps")
                            nc.tensor.matmul(
                                ps[:],
                                kvh[g, h][:, ecl * EC : (ecl + 1) * EC],
                                w_sb[:, g, :],
                                start=True,
                                stop=True,
                            )
                            dst = stage[:, kk * MT : (kk + 1) * MT]
                            if _ev_engine(ev_i) == "A":
                                nc.scalar.activation(dst, ps[:], COPY)
                            else:
                                nc.vector.tensor_copy(dst, ps[:])
                            ev_i += 1
                        nc.sync.dma_start(
                            out_v[g, h * (NECH // KG) + kb],
                            stage[:].rearrange("e (k j) -> e k j", j=MT),
                        )
                        unit += 1
                        if unit == 2:
                            load_kv(0, 1)
                        elif unit == 6:
                            load_kv(1, 0)
                        elif unit == 10:
                            load_kv(1, 1)

    nc.compile()
    return nc


def _get_compiled():
    global _compiled
    if _compiled is None:
        _compiled = _build()
    return _compiled


def _enable_trace_hook():
    """Register the axon NTFF profile hook (missing antenv.axon_hooks shim)."""
    import types

    try:
        import antenv.axon_hooks  # noqa: F401

        return
    except ImportError:
        pass
    try:
        import antenv

        mod = types.ModuleType("antenv.axon_hooks")
        holder = {}
        mod.set_axon_ntff_profile_hook = lambda h: holder.__setitem__("h", h)
        mod.get_axon_ntff_profile_hook = lambda: holder.get("h")
        antenv.axon_hooks = mod
        sys.modules["antenv.axon_hooks"] = mod
        if "/root/.axon_site" not in sys.path:
            sys.path.insert(0, "/root/.axon_site")
        from trn_agent_boot.trn_boot import _ntff_profile_via_ctypes

        mod.set_axon_ntff_profile_hook(
            _ntff_profile_via_ctypes("/opt/axon/libaxon_pjrt.so")
        )

        import concourse.bass_utils as bu

        orig = bu.upload_artifacts

        def _safe_upload(tmpdir):
            try:
                return orig(tmpdir)
            except Exception:
                return tmpdir
    except Exception as e:  # tracing is best-effort
        print(f"trace hook setup failed: {e}")


def kernel(r_idx, r_weight, kv):
    import ml_dtypes

    from concourse.bass_utils import run_bass_kernel_spmd

    bf16 = ml_dtypes.bfloat16

    r_idx = np.asarray(r_idx)
    r_weight = np.asarray(r_weight, dtype=np.float32)
    kv = np.asarray(kv, dtype=np.float32)
    assert r_idx.shape == (N, P2, TOPK) and kv.shape == (N, P2, HW_KV, C_KV)

    nc = _get_compiled()

    cols = np.arange(TPB)
    in_maps = []
    for c in range(NCORES):
        b0 = c * NB
        # kv2[bip*49 + row, g, h, e] = kv[b0 + 2g + bip, row, h*8192 + e]
        kv_c = kv[b0 : b0 + NB].reshape(NPAIR, 2, P2, 2, HROW)
        kvT2 = np.ascontiguousarray(kv_c.transpose(1, 2, 0, 3, 4)).reshape(
            CP, NPAIR, 2, HROW
        )
        idx4 = np.asarray(r_idx[b0 : b0 + NB], dtype=np.int64).reshape(NB, TPB)
        w4 = r_weight[b0 : b0 + NB].reshape(NB, TPB)
        W2 = np.zeros((CP, NPAIR, MT), dtype=np.float32)
        for g in range(NPAIR):
            for bip in range(2):
                b = 2 * g + bip
                W2[bip * P2 + idx4[b], g, bip * TPB + cols] = w4[b]
        in_maps.append({"kv": kvT2.astype(bf16), "w": W2.astype(bf16)})

    trace = bool(int(os.environ.get("KV_TRACE", "0")))
    if trace:
        _enable_trace_hook()
    res = run_bass_kernel_spmd(nc, in_maps, list(range(NCORES)), trace=trace)

    if trace:
        kernel.last_exec_time_ns = res.exec_time_ns
        kernel.last_trace = (
            res.instructions_and_trace[1] if res.instructions_and_trace else None
        )

    out = np.empty((N, P2, TOPK, HW_KV, C_KV), dtype=np.float32)
    for c in range(NCORES):
        b0 = c * NB
        a = np.asarray(res.results[c]["out"]).reshape(NPAIR, ROW // EC, EC, 2, TPB)
        a = a.transpose(0, 3, 4, 1, 2).reshape(NB, TPB, ROW)
        out[b0 : b0 + NB] = a.astype(np.float32).reshape(NB, P2, TOPK, HW_KV, C_KV)
    return out
